# revision 4
# baseline (speedup 1.0000x reference)
"""Trainium2 Bass kernel for nn_Backbone_47390669144486 (SAM-style 4D-correlation attention).

Strategy: data-parallel over the 75 queries across 8 NeuronCores (pad to 80,
10 queries/core).  The entire per-query pipeline (1x1 conv + BN + ReLU,
L2-normalize, 400x400 correlation per way, dual gauss-norm softmax branches,
attention pooling, cosine similarity) is fused on-chip; nothing but the inputs
and the final [way, nq] similarities touch HBM.

Key algebraic moves (all exact up to fp rounding):
  * softmax is shift-invariant -> the gauss-norm mean never needs to be
    subtracted inside exp; only the scale r = 1/(TEMP*sqrt(var+eps)) matters.
  * row/col sums and sum-of-squares of the correlation matrix factor through
    the feature Gram matrices:  sum_kl corr = s^T qrow,
    sum_kl corr^2 = s^T (Q Q^T) s  -> tiny matmuls instead of big reductions.
  * exp + softmax-denominator fuse into one ScalarE activation (accum_out).
  * attention weights are broadcast across 64 partitions by replicating the
    1/S weight vector into the matmul's stationary operand, so the attention
    sums come out of PSUM already partition-broadcast for the pooling step.
  * the /400 means and the 1e-6 norm clamps rescale out of the cosine
    similarity (clamp becomes 4e-4 on the unscaled norms).
"""

import os
import sys

sys.path.insert(0, "/opt/trn_rl_repo")

import numpy as np

import concourse.bass as bass
import concourse.tile as tile
from concourse import bacc, masks, mybir
from concourse.bass_utils import run_bass_kernel_spmd

F32 = mybir.dt.float32
F32R = mybir.dt.float32r
AF = mybir.ActivationFunctionType
OP = mybir.AluOpType
AX = mybir.AxisListType

WAY = 5
C = 64
S = 400          # 20*20 spatial positions
CH = 100         # chunk of the spatial dim that fits PSUM partitions
NCH = S // CH    # 4
NCORES = 8
QPC = 10         # queries per core (75 padded to 80)
TEMP = 5.0


def _build_program():
    nc = bacc.Bacc("TRN2", target_bir_lowering=False, debug=False)

    spt_t = nc.dram_tensor("spt", [WAY, C, S], F32, kind="ExternalInput")
    qry_t = nc.dram_tensor("qry", [QPC, C, S], F32, kind="ExternalInput")
    w_t = nc.dram_tensor("conv_w", [C, C], F32, kind="ExternalInput")
    gam_t = nc.dram_tensor("bn_gamma", [C], F32, kind="ExternalInput")
    bet_t = nc.dram_tensor("bn_beta", [C], F32, kind="ExternalInput")
    mu_t = nc.dram_tensor("bn_mean", [C], F32, kind="ExternalInput")
    var_t = nc.dram_tensor("bn_var", [C], F32, kind="ExternalInput")
    scl_t = nc.dram_tensor("scale", [1], F32, kind="ExternalInput")
    out_t = nc.dram_tensor("out_sim", [WAY, QPC], F32, kind="ExternalOutput")

    from contextlib import ExitStack

    with tile.TileContext(nc) as tc, ExitStack() as ctx:
        _emit(ctx, tc, nc, spt_t.ap(), qry_t.ap(), w_t.ap(), gam_t.ap(),
              bet_t.ap(), mu_t.ap(), var_t.ap(), scl_t.ap(), out_t.ap())
    nc.compile()
    _dedup_act_table_loads(nc)
    return nc


def _dedup_act_table_loads(nc):
    """The act-table pass alternates natural_log / exp_and_others per function.
    natural_log_exp_and_others serves every activation this kernel uses
    (Exp, Ln, Relu), so keep one load targeting it and drop the rest."""
    from concourse.hw_specs import get_activation_tables

    names = list(get_activation_tables(nc.m.arch).keys())
    combined = names.index("natural_log_exp_and_others")
    kept = False
    for b in nc.m.functions[0].blocks:
        keep = []
        for i in b.instructions:
            if type(i).__name__ == "InstLoadActFuncSet":
                si = i.sync_info
                assert si is None or (not si.on_wait and not si.on_update)
                if kept:
                    continue
                i.act_func_set_id = combined
                kept = True
            keep.append(i)
        if len(keep) != len(b.instructions):
            b.instructions[:] = keep


def _emit(ctx, tc, nc, spt, qry, conv_w, gam, bet, mu, var, scl, out_sim):
    STAGE = int(os.environ.get("BASS_KSTAGE", "99"))

    def mmr(out, lhsT, rhs, **kw):
        # float32r streams the moving operand at 1 col/cycle (vs 4 for fp32)
        # once N >= 256; all call sites here have N = 400.
        nc.tensor.matmul(out=out, lhsT=lhsT.bitcast(F32R), rhs=rhs.bitcast(F32R),
                         **kw)

    consts = ctx.enter_context(tc.tile_pool(name="consts", bufs=1))
    pre = ctx.enter_context(tc.tile_pool(name="pre", bufs=2))
    perq = ctx.enter_context(tc.tile_pool(name="perq", bufs=2))
    epool = ctx.enter_context(tc.tile_pool(name="epool", bufs=10))
    wpool = ctx.enter_context(tc.tile_pool(name="wpool", bufs=6))
    # PSUM: exactly 8 banks total
    ps_corr = ctx.enter_context(tc.tile_pool(name="ps_corr", bufs=4, space="PSUM"))
    ps_stat = ctx.enter_context(tc.tile_pool(name="ps_stat", bufs=2, space="PSUM"))
    ps_big = ctx.enter_context(tc.tile_pool(name="ps_big", bufs=1, space="PSUM"))
    ps_tp = ctx.enter_context(tc.tile_pool(name="ps_tp", bufs=1, space="PSUM"))

    # ---- constants ----
    ident = consts.tile([128, 128], F32)
    masks.make_identity(nc, ident[:])
    ones128 = consts.tile([128, 1], F32)
    nc.gpsimd.memset(ones128[:], 1.0)
    oinv_rep = consts.tile([C, C], F32)          # all 1/64 -> mean-broadcast matmul
    nc.gpsimd.memset(oinv_rep[:], 1.0 / C)
    ones_rep = consts.tile([128, C], F32)        # for free-dim broadcast of [P,1] vecs
    nc.gpsimd.memset(ones_rep[:], 1.0)
    zeros = consts.tile([128, 1], F32)
    nc.gpsimd.memset(zeros[:], 0.0)
    c25e5 = consts.tile([128, 1], F32)           # bias for stats sqrt: 25*1e-5
    nc.gpsimd.memset(c25e5[:], 25.0e-5)
    c1e5 = consts.tile([128, 1], F32)            # bias for BN sqrt: 1e-5
    nc.gpsimd.memset(c1e5[:], 1.0e-5)

    # ---- input loads ----
    spt_raw = consts.tile([C, WAY * S], F32)
    nc.sync.dma_start(out=spt_raw[:].rearrange("c (w s) -> c w s", w=WAY),
                      in_=spt.rearrange("w c s -> c w s"))
    qry_raw = consts.tile([C, QPC * S], F32)
    nc.sync.dma_start(out=qry_raw[:].rearrange("c (q s) -> c q s", q=QPC),
                      in_=qry.rearrange("q c s -> c q s"))
    w_sb = consts.tile([C, C], F32)
    nc.sync.dma_start(out=w_sb[:], in_=conv_w)
    gam_sb = consts.tile([C, 1], F32)
    nc.sync.dma_start(out=gam_sb[:], in_=gam.unsqueeze(1))
    bet_sb = consts.tile([C, 1], F32)
    nc.sync.dma_start(out=bet_sb[:], in_=bet.unsqueeze(1))
    mu_sb = consts.tile([C, 1], F32)
    nc.sync.dma_start(out=mu_sb[:], in_=mu.unsqueeze(1))
    var_sb = consts.tile([C, 1], F32)
    nc.sync.dma_start(out=var_sb[:], in_=var.unsqueeze(1))
    scale_b = consts.tile([WAY, 1], F32)
    nc.gpsimd.dma_start(
        out=scale_b[:],
        in_=bass.AP(tensor=scl.tensor, offset=scl.offset, ap=[[0, WAY], [1, 1]]))

    # conv weight transposed: lhsT layout [c_in, c_out]
    wT_ps = ps_tp.tile([C, C], F32, tag="tp")
    nc.tensor.transpose(wT_ps[:], w_sb[:], ident[0:C, 0:C])
    wT_sb = consts.tile([C, C], F32)
    nc.vector.tensor_copy(wT_sb[:], wT_ps[:])

    # BN fold: bns = gamma / sqrt(var + 1e-5);  bnb = beta - mean * bns
    sd = pre.tile([C, 1], F32, tag="bn")
    nc.scalar.activation(sd[:], var_sb[:], AF.Ln, bias=c1e5[0:C], scale=1.0)
    rsd = pre.tile([C, 1], F32, tag="bn")
    nc.scalar.activation(rsd[:], sd[:], AF.Exp, bias=zeros[0:C], scale=-0.5)
    bns = consts.tile([C, 1], F32)
    nc.vector.tensor_mul(bns[:], gam_sb[:], rsd[:])
    mb = pre.tile([C, 1], F32, tag="bn")
    nc.vector.tensor_mul(mb[:], mu_sb[:], bns[:])
    bnb = consts.tile([C, 1], F32)
    nc.vector.scalar_tensor_tensor(out=bnb[:], in0=mb[:], scalar=-1.0,
                                   in1=bet_sb[:], op0=OP.mult, op1=OP.add)

    # ---- channel-mean centering (normalize_feature) ----
    spt_n = consts.tile([C, WAY * S], F32)
    for w in range(WAY):
        mean_ps = ps_big.tile([C, S], F32, tag="big")
        mmr(mean_ps[:], oinv_rep[:],
            spt_raw[:, w * S:(w + 1) * S], start=True, stop=True)
        nc.vector.tensor_sub(spt_n[:, w * S:(w + 1) * S],
                             spt_raw[:, w * S:(w + 1) * S], mean_ps[:])
    qry_n = consts.tile([C, QPC * S], F32)
    for q in range(QPC):
        mean_ps = ps_big.tile([C, S], F32, tag="big")
        mmr(mean_ps[:], oinv_rep[:],
            rhs=qry_raw[:, q * S:(q + 1) * S], start=True, stop=True)
        nc.vector.tensor_sub(qry_n[:, q * S:(q + 1) * S],
                             qry_raw[:, q * S:(q + 1) * S], mean_ps[:])

    # ---- feature transform helper: conv+bn+relu then L2-normalize columns ----
    def feat_transform(x_slice, h_sb, hT_sc, G_sb, row_accum_col, rowp_tile):
        """x_slice [64,400] -> h_sb [64,400] (L2-normalized), hT_sc [100, 256]
        (normalized transpose chunks), G_sb [64,64] Gram, rowp accum cols."""
        y_ps = ps_big.tile([C, S], F32, tag="big")
        nc.tensor.matmul(out=y_ps[:], lhsT=wT_sb[:], rhs=x_slice, start=True, stop=True)
        bnr = pre.tile([C, S], F32, tag="bnr")
        nc.scalar.activation(bnr[:], y_ps[:], AF.Relu, bias=bnb[:], scale=bns[:])
        hT_raw = pre.tile([CH, NCH * C], F32, tag="hTraw")
        nsq = pre.tile([CH, NCH], F32, tag="nsq")
        for j in range(NCH):
            tp_ps = ps_tp.tile([CH, C], F32, tag="tp")
            nc.tensor.transpose(tp_ps[:], bnr[:, j * CH:(j + 1) * CH], ident[0:C, 0:C])
            nc.vector.tensor_copy(hT_raw[:, j * C:(j + 1) * C], tp_ps[:])
            sqscr = pre.tile([CH, C], F32, tag="sqscr")
            nc.vector.scalar_tensor_tensor(
                out=sqscr[:], in0=hT_raw[:, j * C:(j + 1) * C], scalar=1.0,
                in1=hT_raw[:, j * C:(j + 1) * C],
                op0=OP.mult, op1=OP.mult, accum_out=nsq[:, j:j + 1])
        nc.vector.tensor_scalar_max(nsq[:], nsq[:], 1.0e-16)
        nrm = pre.tile([CH, NCH], F32, tag="nrm")
        nc.scalar.activation(nrm[:], nsq[:], AF.Ln, bias=zeros[0:CH], scale=1.0)
        rinv = pre.tile([CH, NCH], F32, tag="rinv")
        nc.scalar.activation(rinv[:], nrm[:], AF.Exp, bias=zeros[0:CH], scale=-0.5)
        for j in range(NCH):
            nc.vector.tensor_scalar_mul(hT_sc[:, j * C:(j + 1) * C],
                                        hT_raw[:, j * C:(j + 1) * C], rinv[:, j:j + 1])
        G_ps = ps_stat.tile([C, C], F32, tag="stat")
        for j in range(NCH):
            nc.tensor.matmul(out=G_ps[:], lhsT=hT_sc[:, j * C:(j + 1) * C],
                             rhs=hT_sc[:, j * C:(j + 1) * C],
                             start=(j == 0), stop=(j == NCH - 1))
        nc.vector.tensor_copy(G_sb[:], G_ps[:])
        for j in range(NCH):
            bk_ps = ps_tp.tile([C, CH], F32, tag="tp")
            nc.tensor.transpose(bk_ps[:], hT_sc[:, j * C:(j + 1) * C], ident[0:CH, 0:CH])
            nc.vector.tensor_scalar(
                out=h_sb[:, j * CH:(j + 1) * CH], in0=bk_ps[:], scalar1=1.0,
                scalar2=None, op0=OP.mult, op1=OP.add,
                accum_out=rowp_tile[:, row_accum_col + j:row_accum_col + j + 1])

    # ---- support features (once) ----
    if STAGE < 2:
        sims = consts.tile([WAY, QPC], F32)
        nc.gpsimd.memset(sims[:], 0.0)
        nc.vector.tensor_copy(sims[0:WAY, 0:1], qry_n[0:WAY, 0:1])
        nc.sync.dma_start(out=out_sim, in_=sims[:])
        return
    sh = consts.tile([128, WAY * S], F32)        # \hat{s}; rows 64-127 = copy
    Gw_sb = consts.tile([128, WAY * C], F32)
    srowp = consts.tile([C, WAY * NCH], F32)
    for w in range(WAY):
        shT = pre.tile([CH, NCH * C], F32, tag="shT")
        feat_transform(spt_n[:, w * S:(w + 1) * S], sh[0:C, w * S:(w + 1) * S],
                       shT, Gw_sb[0:C, w * C:(w + 1) * C], w * NCH, srowp)
    srow = consts.tile([128, WAY], F32)
    nc.vector.tensor_reduce(out=srow[0:C, :],
                            in_=srowp[:].rearrange("c (w j) -> c w j", w=WAY),
                            axis=AX.X, op=OP.add)
    # duplicate support operands into partitions 64-127 for PE row-packing
    nc.sync.dma_start(out=sh[C:128, :], in_=sh[0:C, :])
    nc.sync.dma_start(out=Gw_sb[C:128, :], in_=Gw_sb[0:C, :])
    nc.sync.dma_start(out=srow[C:128, :], in_=srow[0:C, :])

    sims = consts.tile([WAY, QPC], F32)
    nc.gpsimd.memset(sims[:], 0.0)

    # ---- per-query pipeline ----
    for q in range(QPC if STAGE >= 3 else 0):
        qn_sl = qry_n[:, q * S:(q + 1) * S]
        qh = perq.tile([128, S], F32, tag="qh")
        qhT = perq.tile([CH, NCH * C], F32, tag="qhT")
        Gq_sb = perq.tile([C, C], F32, tag="Gq")
        qrowp = perq.tile([C, NCH], F32, tag="qrowp")
        feat_transform(qn_sl, qh[0:C, :], qhT, Gq_sb, 0, qrowp)
        nc.sync.dma_start(out=qh[C:128, :], in_=qh[0:C, :])
        qrow = perq.tile([C, 1], F32, tag="qrow")
        nc.vector.tensor_reduce(out=qrow[:], in_=qrowp[:], axis=AX.X, op=OP.add)

        spt_att = perq.tile([C, WAY], F32, tag="spt_att")
        qry_att = perq.tile([C, WAY], F32, tag="qry_att")

        for w in range(WAY):
            s_lo = sh[0:C, w * S:(w + 1) * S]
            s_hi = sh[C:128, w * S:(w + 1) * S]
            qh_lo = qh[0:C, :]
            qh_hi = qh[C:128, :]
            # --- factored stats; U (rows 0-63) and T (rows 64-127) run
            # concurrently in the PE array via row/col tiling ---
            U_ps = ps_corr.tile([128, S], F32, tag="corr")
            nc.tensor.matmul(out=U_ps[0:C, :], lhsT=Gq_sb[:], rhs=s_lo,
                             start=True, stop=True)
            T_ps = ps_corr.tile([128, S], F32, tag="corr")
            nc.tensor.matmul(out=T_ps[C:128, :],
                             lhsT=Gw_sb[C:128, w * C:(w + 1) * C], rhs=qh_hi,
                             start=True, stop=True, tile_position=(C, C))
            V12 = perq.tile([128, S], F32, tag="V12")
            nc.vector.tensor_mul(V12[0:C, :], U_ps[0:C, :], s_lo)
            nc.vector.tensor_mul(V12[C:128, :], T_ps[C:128, :], qh_hi)

            st1 = ps_stat.tile([CH, 8], F32, tag="stat")   # branch A: S1 0:4, S2 4:8
            st2 = ps_stat.tile([CH, 8], F32, tag="stat")   # branch B
            for j in range(NCH):
                nc.tensor.matmul(out=st1[:, j:j + 1],
                                 lhsT=s_lo[:, j * CH:(j + 1) * CH], rhs=qrow[:],
                                 start=True, stop=True)
                nc.tensor.matmul(out=st2[:, j:j + 1],
                                 lhsT=qh_hi[:, j * CH:(j + 1) * CH],
                                 rhs=srow[C:128, w:w + 1],
                                 start=True, stop=True, tile_position=(C, 0))
                nc.tensor.matmul(out=st1[:, 4 + j:5 + j],
                                 lhsT=V12[0:C, j * CH:(j + 1) * CH],
                                 rhs=ones128[0:C], start=True, stop=True)
                nc.tensor.matmul(out=st2[:, 4 + j:5 + j],
                                 lhsT=V12[C:128, j * CH:(j + 1) * CH],
                                 rhs=ones128[C:128],
                                 start=True, stop=True, tile_position=(C, 0))
            # r = 1 / (TEMP * sqrt(var + 1e-5)),  var = (S2 - S1^2/400)/399
            s1c = perq.tile([CH, 8], F32, tag="s1c")
            nc.vector.tensor_copy(s1c[:, 0:4], st1[:, 0:4])
            nc.vector.tensor_copy(s1c[:, 4:8], st2[:, 0:4])
            tsq = perq.tile([CH, 8], F32, tag="tsq")
            nc.vector.tensor_mul(tsq[:, 0:4], s1c[:, 0:4], st1[:, 0:4])
            nc.vector.tensor_mul(tsq[:, 4:8], s1c[:, 4:8], st2[:, 0:4])
            usq = perq.tile([CH, 8], F32, tag="usq")
            nc.vector.scalar_tensor_tensor(out=usq[:, 0:4], in0=tsq[:, 0:4],
                                           scalar=-1.0 / S, in1=st1[:, 4:8],
                                           op0=OP.mult, op1=OP.add)
            nc.vector.scalar_tensor_tensor(out=usq[:, 4:8], in0=tsq[:, 4:8],
                                           scalar=-1.0 / S, in1=st2[:, 4:8],
                                           op0=OP.mult, op1=OP.add)
            sq = perq.tile([CH, 8], F32, tag="sq")
            nc.scalar.activation(sq[:], usq[:], AF.Ln, bias=c25e5[0:CH],
                                 scale=(TEMP * TEMP) / (S - 1.0))
            rr = perq.tile([CH, 8], F32, tag="rr")
            nc.scalar.activation(rr[:], sq[:], AF.Exp, bias=zeros[0:CH], scale=-0.5)

            # --- correlation pairs: branch A (rows 0-63) || branch B (64-127) ---
            srowE = perq.tile([CH, 8], F32, tag="srowE")
            EA = []
            EB = []
            for j in range(NCH):
                cpa = ps_corr.tile([CH, S], F32, tag="corr")
                nc.tensor.matmul(out=cpa[:], lhsT=s_lo[:, j * CH:(j + 1) * CH],
                                 rhs=qh_lo, start=True, stop=True)
                cpb = ps_corr.tile([CH, S], F32, tag="corr")
                nc.tensor.matmul(out=cpb[:], lhsT=qh_hi[:, j * CH:(j + 1) * CH],
                                 rhs=s_hi, start=True, stop=True,
                                 tile_position=(C, 0))
                ea = epool.tile([CH, S], F32, tag="E")
                nc.scalar.activation(ea[:], cpa[:], AF.Exp, bias=zeros[0:CH],
                                     scale=rr[:, j:j + 1],
                                     accum_out=srowE[:, j:j + 1])
                EA.append(ea)
                eb = epool.tile([CH, S], F32, tag="E")
                nc.scalar.activation(eb[:], cpb[:], AF.Exp, bias=zeros[0:CH],
                                     scale=rr[:, 4 + j:5 + j],
                                     accum_out=srowE[:, 4 + j:5 + j])
                EB.append(eb)
            wrec = perq.tile([CH, 8], F32, tag="wrec")
            nc.vector.reciprocal(wrec[:], srowE[:])
            if STAGE < 4:
                nc.vector.tensor_copy(spt_att[:, w:w + 1], wrec[0:C, 0:1])
                nc.vector.tensor_copy(qry_att[:, w:w + 1], wrec[0:C, 1:2])
                continue

            # --- attention sums, partition-broadcast via replicated weights ---
            attA = ps_big.tile([C, S], F32, tag="big")   # = attn_q broadcast [64,400]
            for j in range(NCH):
                wrt = wpool.tile([CH, C], F32, tag="wrt")
                nc.gpsimd.tensor_scalar_mul(wrt[:], ones_rep[0:CH, :], wrec[:, j:j + 1])
                nc.tensor.matmul(out=attA[:], lhsT=wrt[:], rhs=EA[j][:],
                                 start=(j == 0), stop=(j == NCH - 1))
            pj = perq.tile([C, S], F32, tag="pj")
            nc.vector.scalar_tensor_tensor(out=pj[:], in0=qn_sl, scalar=1.0,
                                           in1=attA[:], op0=OP.mult, op1=OP.mult,
                                           accum_out=qry_att[:, w:w + 1])
            attB = ps_big.tile([C, S], F32, tag="big")   # = attn_s broadcast
            for j in range(NCH):
                wrt = wpool.tile([CH, C], F32, tag="wrt")
                nc.gpsimd.tensor_scalar_mul(wrt[:], ones_rep[0:CH, :], wrec[:, 4 + j:5 + j])
                nc.tensor.matmul(out=attB[:], lhsT=wrt[:], rhs=EB[j][:],
                                 start=(j == 0), stop=(j == NCH - 1))
            pj2 = perq.tile([C, S], F32, tag="pj")
            nc.vector.scalar_tensor_tensor(out=pj2[:], in0=spt_n[:, w * S:(w + 1) * S],
                                           scalar=1.0, in1=attB[:], op0=OP.mult,
                                           op1=OP.mult, accum_out=spt_att[:, w:w + 1])

        # --- cosine similarity over channels (partition dim) via ones-matmul ---
        P3 = perq.tile([C, 3 * WAY], F32, tag="P3")
        nc.vector.tensor_mul(P3[:, 0:WAY], spt_att[:], qry_att[:])
        nc.vector.tensor_mul(P3[:, WAY:2 * WAY], spt_att[:], spt_att[:])
        nc.vector.tensor_mul(P3[:, 2 * WAY:3 * WAY], qry_att[:], qry_att[:])
        dots = ps_stat.tile([WAY, 3], F32, tag="stat")
        for i in range(3):
            nc.tensor.matmul(out=dots[:, i:i + 1], lhsT=P3[:, i * WAY:(i + 1) * WAY],
                             rhs=ones128[0:C], start=True, stop=True)
        nrm2 = perq.tile([WAY, 2], F32, tag="nrm2")
        nc.vector.tensor_scalar_max(nrm2[:], dots[:, 1:3], 1.6e-7)
        lnn = perq.tile([WAY, 2], F32, tag="lnn")
        nc.scalar.activation(lnn[:], nrm2[:], AF.Ln, bias=zeros[0:WAY], scale=1.0)
        lsum = perq.tile([WAY, 1], F32, tag="lsum")
        nc.vector.tensor_add(lsum[:], lnn[:, 0:1], lnn[:, 1:2])
        rden = perq.tile([WAY, 1], F32, tag="rden")
        nc.scalar.activation(rden[:], lsum[:], AF.Exp, bias=zeros[0:WAY], scale=-0.5)
        s0 = perq.tile([WAY, 1], F32, tag="s0")
        nc.vector.tensor_mul(s0[:], dots[:, 0:1], rden[:])
        nc.vector.tensor_mul(sims[:, q:q + 1], s0[:], scale_b[:])

    nc.sync.dma_start(out=out_sim, in_=sims[:])


_PROGRAM = None


def _get_program():
    global _PROGRAM
    if _PROGRAM is None:
        _PROGRAM = _build_program()
    return _PROGRAM


def kernel(spt, qry, conv_w, bn_gamma, bn_beta, bn_mean, bn_var, scale):
    spt = np.ascontiguousarray(np.asarray(spt, dtype=np.float32).reshape(WAY, C, S))
    qry = np.asarray(qry, dtype=np.float32).reshape(-1, C, S)
    nq = qry.shape[0]
    npad = NCORES * QPC
    qpad = np.zeros((npad, C, S), dtype=np.float32)
    qpad[:nq] = qry
    in_maps = []
    for core in range(NCORES):
        in_maps.append({
            "spt": spt,
            "qry": np.ascontiguousarray(qpad[core * QPC:(core + 1) * QPC]),
            "conv_w": np.asarray(conv_w, dtype=np.float32),
            "bn_gamma": np.asarray(bn_gamma, dtype=np.float32),
            "bn_beta": np.asarray(bn_beta, dtype=np.float32),
            "bn_mean": np.asarray(bn_mean, dtype=np.float32),
            "bn_var": np.asarray(bn_var, dtype=np.float32),
            "scale": np.asarray(scale, dtype=np.float32),
        })
    nc = _get_program()
    trace = bool(os.environ.get("KBENCH_TRACE"))
    kw = {}
    if trace:
        import tempfile
        kw = dict(trace=True, tmpdir=tempfile.mkdtemp(prefix="ktrace_"))
    res = run_bass_kernel_spmd(nc, in_maps, list(range(NCORES)), **kw)
    if trace:
        global LAST_RESULTS
        LAST_RESULTS = res
        print("exec_time_ns:", res.exec_time_ns,
              "mean:", res.mean_exec_time_ns,
              "worst core:", res.max_exec_time_core_id)
        if res.instructions_and_trace:
            print("trace path:", res.instructions_and_trace[1])
    outs = [np.asarray(res.results[i]["out_sim"]) for i in range(NCORES)]
    full = np.concatenate([o.T for o in outs], axis=0)  # [80, WAY]
    return np.ascontiguousarray(full[:nq]).astype(np.float32)


if __name__ == "__main__":
    rng = np.random.default_rng(0)
    ins = {
        "spt": rng.standard_normal((WAY, C, 20, 20), dtype=np.float32),
        "qry": rng.standard_normal((75, C, 20, 20), dtype=np.float32),
        "conv_w": (rng.standard_normal((C, C)) * 0.1).astype(np.float32),
        "bn_gamma": np.ones(C, np.float32),
        "bn_beta": np.zeros(C, np.float32),
        "bn_mean": np.zeros(C, np.float32),
        "bn_var": np.ones(C, np.float32),
        "scale": np.ones(1, np.float32),
    }
    out = kernel(**ins)
    print(out.shape, out.dtype, out[:2])

